# revision 1
# baseline (speedup 1.0000x reference)
"""AttendAndSpell (LAS decoder) Trainium2 Bass kernel.

Strategy: pure data-parallel over batch (B=64 -> 8 items/core, 8 cores), no
collectives.

Layouts (per core, items b = 0..7):
  - "strided batch": item b lives on SBUF partition PB[b], PB = [0, 32, 64,
    96, 16, 48, 80, 112] (compute engines cannot address partition offsets
    that are not 32-aligned, and PE column groups are 32-aligned; merging the
    two 4-item groups at a 16-partition stride gives all 8 items in one tile).
    All states / gates / softmax tensors are [128, *] tiles, elementwise ops
    run full-tile (garbage rows are zero/bounded and never read).
  - transposed fp16 stationaries [128, kt, 128] via PE transposes: item b in
    column PB[b] (transpose of the strided rows), used as matmul lhsT.
  - attention operands: hp in two layouts (r-on-partition, d-on-partition),
    built once in phase 1; per-item score/context matmuls (M=1) are packed
    4-wide across PE column groups via tile_position.

Per step: gate matmuls are activation-stationary (fp16 weights moving, SBUF
resident); output projection is deferred (s1/c stream to DRAM as fp16,
one [T*8, 1024] @ [1024, 4096] GEMM at the end). All matmul operands fp16
(fp32 PSUM accumulation); elementwise/softmax/bias/embedding math fp32.
Measured numpy end-to-end error of the fp16 scheme: ~4e-4 rel, flat in t.
"""

import math

import numpy as np

import concourse.bacc as bacc
import concourse.mybir as mybir
import concourse.tile as tile
from concourse.bass_utils import run_bass_kernel_spmd
from concourse.masks import make_identity

B, R, T, H, V = 64, 256, 128, 512, 4096
NCORES = 8
BS = B // NCORES  # 8
G = 4 * H  # 2048
KC = 2 * H  # 1024
KT_H = H // 128  # 4
KT_KC = KC // 128  # 8
RT = R // 128  # 2
F32 = mybir.dt.float32
F16 = mybir.dt.float16
AF = mybir.ActivationFunctionType
ALU = mybir.AluOpType
AX = mybir.AxisListType

PB = [0, 32, 64, 96, 16, 48, 80, 112]  # partition of item b
PERM16 = [0, 4, 1, 5, 2, 6, 3, 7]  # row 16*j holds item PERM16[j]
INV16 = [0, 2, 4, 6, 1, 3, 5, 7]  # col INV16[b] (16-stride) holds item b


def _s16(ap):
    """[128, ...] -> 16-stride partition view [8, ...] (rows 0,16,...,112)."""
    return ap.rearrange("(i s) ... -> i s ...", s=16)[:, 0]


def build_program(T_steps=T):
    nc = bacc.Bacc(None, target_bir_lowering=False)

    d_hT = nc.dram_tensor("hT", [H, BS * R], F16, kind="ExternalInput")
    d_W0T = nc.dram_tensor("W0T", [KC, G], F16, kind="ExternalInput")
    d_W1T = nc.dram_tensor("W1T", [KC, G], F16, kind="ExternalInput")
    d_phiT = nc.dram_tensor("phiT", [H, H], F16, kind="ExternalInput")
    d_psiT = nc.dram_tensor("psiT", [H, H], F16, kind="ExternalInput")
    d_psib_bc = nc.dram_tensor("psib_bc", [128, H], F32, kind="ExternalInput")
    d_psibT = nc.dram_tensor("psibT", [128, KT_H], F32, kind="ExternalInput")
    d_phibT = nc.dram_tensor("phibT", [128, KT_H], F16, kind="ExternalInput")
    d_b1_bc = nc.dram_tensor("b1_bc", [128, G], F32, kind="ExternalInput")
    # embb rows are in PERM16 item order (row j -> partition 16*j)
    d_embb = nc.dram_tensor("embb", [T_steps, BS, G], F32, kind="ExternalInput")
    d_owT = nc.dram_tensor("owT", [KC, V], F16, kind="ExternalInput")
    d_smat = nc.dram_tensor("smat", [128, 128], F16, kind="ExternalInput")
    d_ob_bc = nc.dram_tensor("ob_bc", [128, V], F32, kind="ExternalInput")
    # out[t, j, :] is item PERM16[j]; host unscrambles + transposes
    d_out = nc.dram_tensor("out", [T_steps, BS, V], F32, kind="ExternalOutput")
    d_histT = nc.dram_tensor("histT", [T_steps, 128, KT_KC, BS], F16)

    with tile.TileContext(nc) as tc:
        with (
            tc.tile_pool(name="persist", bufs=1) as persist,
            tc.tile_pool(name="work", bufs=2) as work,
            tc.tile_pool(name="workbig", bufs=1) as workbig,
        ):
            ident = persist.tile([128, 128], F32)
            make_identity(nc, ident)

            # strided-batch fp32 state tiles
            s0 = persist.tile([128, H], F32)
            s1 = persist.tile([128, H], F32)
            cs0 = persist.tile([128, H], F32)
            cs1 = persist.tile([128, H], F32)
            cstr0 = persist.tile([128, H], F32)  # context, items 0-3 at rows 32j
            cstr1 = persist.tile([128, H], F32)  # context, items 4-7 at rows 32j
            # transposed fp16 stationaries [128, kt, col]
            s0T = persist.tile([128, KT_H, 128], F16)
            s1T = persist.tile([128, KT_H, 128], F16)
            cT = persist.tile([128, KT_H, 128], F16)
            spT = persist.tile([128, KT_H, BS], F16)  # col j = item PERM16[j]
            alT0 = persist.tile([128, RT, 128], F16)
            alT1 = persist.tile([128, RT, 128], F16)
            for st in (s0, s1, cs0, cs1, cstr0, cstr1):
                nc.vector.memset(st, 0.0)
            for st in (s0T, s1T, cT):
                nc.vector.memset(st, 0.0)

            sb_b1 = persist.tile([128, G], F32)
            nc.sync.dma_start(sb_b1, d_b1_bc[:])
            sb_smat = persist.tile([128, 128], F16)
            nc.sync.dma_start(sb_smat, d_smat[:])
            sb_psibT = persist.tile([128, KT_H], F32)
            nc.sync.dma_start(sb_psibT, d_psibT[:])
            sb_phibT = persist.tile([128, KT_H], F16)
            nc.sync.dma_start(sb_phibT, d_phibT[:])
            ring = [
                persist.tile([128, G], F32, name=f"ring{i}", tag=f"ring{i}")
                for i in range(3)
            ]
            for rg in ring:
                nc.vector.memset(rg, 0.0)

            with tc.tile_pool(name="wts", bufs=1) as wts:
                sb_W0T = wts.tile([128, KT_KC, G], F16)
                nc.sync.dma_start(sb_W0T, d_W0T.rearrange("(kt p) g -> p kt g", p=128))
                sb_W1T = wts.tile([128, KT_KC, G], F16)
                nc.sync.dma_start(sb_W1T, d_W1T.rearrange("(kt p) g -> p kt g", p=128))
                sb_phiT = wts.tile([128, KT_H, H], F16)
                nc.sync.dma_start(sb_phiT, d_phiT.rearrange("(kt p) f -> p kt f", p=128))
                sb_hp = wts.tile([128, RT * BS, H], F16)  # [p, rt*BS+b, d]
                sb_hpT = wts.tile([128, KT_H * BS, R], F16)  # [p, dt*BS+b, r]
                sb_eb0 = wts.tile([128, R], F32)  # e_base items 0-3 at rows 32j
                sb_eb1 = wts.tile([128, R], F32)

                # ---------------- Phase 1 ----------------
                with (
                    tc.tile_pool(name="ph1", bufs=1) as ph1,
                    tc.tile_pool(name="pp1", bufs=2, space="PSUM") as pp1,
                ):
                    NBR = BS * R  # 2048
                    sb_hT = ph1.tile([128, KT_H, NBR], F16)
                    nc.sync.dma_start(sb_hT, d_hT.rearrange("(kt p) n -> p kt n", p=128))
                    sb_psiT = ph1.tile([128, KT_H, H], F16)
                    nc.sync.dma_start(
                        sb_psiT, d_psiT.rearrange("(kt p) f -> p kt f", p=128)
                    )
                    sb_psib = ph1.tile([128, H], F32)
                    nc.sync.dma_start(sb_psib, d_psib_bc[:])

                    # hp (r-on-partition): act-stationary GEMM
                    for m in range(NBR // 128):  # 16
                        ps = pp1.tile([128, H], F32, tag="pp1", name="ps1")
                        for kt in range(KT_H):
                            nc.tensor.matmul(
                                ps,
                                lhsT=sb_hT[:, kt, m * 128 : (m + 1) * 128],
                                rhs=sb_psiT[:, kt, :],
                                start=(kt == 0),
                                stop=(kt == KT_H - 1),
                            )
                        b_, rt_ = divmod(m, RT)
                        nc.vector.tensor_add(sb_hp[:, rt_ * BS + b_, :], ps, sb_psib)
                    # hpT (d-on-partition): weight-stationary GEMM
                    for mt in range(KT_H):
                        for nch in range(NBR // 512):  # 4
                            ps = pp1.tile([128, H], F32, tag="pp1", name="ps2")
                            for kt in range(KT_H):
                                nc.tensor.matmul(
                                    ps,
                                    lhsT=sb_psiT[:, kt, mt * 128 : (mt + 1) * 128],
                                    rhs=sb_hT[:, kt, nch * 512 : (nch + 1) * 512],
                                    start=(kt == 0),
                                    stop=(kt == KT_H - 1),
                                )
                            for j in range(512 // R):  # 2 items per chunk
                                b_ = nch * 2 + j
                                nc.vector.tensor_scalar_add(
                                    sb_hpT[:, mt * BS + b_, :],
                                    ps[:, j * R : (j + 1) * R],
                                    sb_psibT[:, mt : mt + 1],
                                )
                    # e_base: item b -> psum tile b//4, row 32*(b%4)
                    peb0 = pp1.tile([128, H], F32, tag="pp1", name="peb0")
                    peb1 = pp1.tile([128, H], F32, tag="pp1", name="peb1")
                    nc.vector.memset(peb0, 0.0)
                    nc.vector.memset(peb1, 0.0)
                    for b_ in range(BS):
                        ps = peb0 if b_ < 4 else peb1
                        j = (b_ % 4) * 32
                        for dt in range(KT_H):
                            nc.tensor.matmul(
                                ps[j : j + 1, :R],
                                lhsT=sb_phibT[:, dt : dt + 1],
                                rhs=sb_hpT[:, dt * BS + b_, :],
                                start=(dt == 0),
                                stop=(dt == KT_H - 1),
                                tile_position=(0, j),
                            )
                    nc.vector.tensor_copy(sb_eb0, peb0[:, :R])
                    nc.vector.tensor_copy(sb_eb1, peb1[:, :R])

                _p2cms = [
                    tc.tile_pool(name="ptrans", bufs=2, space="PSUM"),
                    tc.tile_pool(name="pgate", bufs=2, space="PSUM"),
                    tc.tile_pool(name="pgatep", bufs=2, space="PSUM"),
                    tc.tile_pool(name="pattn", bufs=2, space="PSUM"),
                ]
                ptrans, pgate, pgatep, pattn = [cm.__enter__() for cm in _p2cms]

                for tpre in range(min(2, T_steps)):
                    nc.sync.dma_start(_s16(ring[tpre]), d_embb[tpre])

                def softmax_half(ef, rc):
                    """softmax over free dim of one [128, R] group (rows 32j)."""
                    mx = work.tile([128, 1], F32, tag="mx")
                    nc.vector.tensor_reduce(mx, ef, AX.X, ALU.max)
                    nc.vector.tensor_scalar_mul(mx, mx, -1.0)
                    al = work.tile([128, R], F32, tag="al")
                    nc.scalar.activation(al, ef, AF.Exp, bias=mx)
                    sm = work.tile([128, 1], F32, tag="sm")
                    nc.vector.tensor_reduce(sm, al, AX.X, ALU.add)
                    nc.vector.reciprocal(rc, sm)
                    return al

                def attention(t):
                    """score -> softmax -> context -> cT; t == -1 uses e_base."""
                    if t >= 0:
                        pe0 = pattn.tile([128, H], F32, tag="pa", name="pe0")
                        pe1 = pattn.tile([128, H], F32, tag="pa", name="pe1")
                        nc.vector.memset(pe0[:, :R], 0.0)
                        nc.vector.memset(pe1[:, :R], 0.0)
                        for b_ in range(BS):
                            ps = pe0 if b_ < 4 else pe1
                            j = (b_ % 4) * 32
                            for dt in range(KT_H):
                                nc.tensor.matmul(
                                    ps[j : j + 1, :R],
                                    lhsT=spT[:, dt, INV16[b_] : INV16[b_] + 1],
                                    rhs=sb_hpT[:, dt * BS + b_, :],
                                    start=(dt == 0),
                                    stop=(dt == KT_H - 1),
                                    tile_position=(0, j),
                                )
                        ef0 = work.tile([128, R], F32, tag="ef0")
                        ef1 = work.tile([128, R], F32, tag="ef1")
                        nc.vector.tensor_add(ef0, pe0[:, :R], sb_eb0)
                        nc.vector.tensor_add(ef1, pe1[:, :R], sb_eb1)
                    else:
                        ef0, ef1 = sb_eb0, sb_eb1
                    rc0 = work.tile([128, 1], F32, tag="rc0")
                    rc1 = work.tile([128, 1], F32, tag="rc1")
                    al0 = softmax_half(ef0, rc0)
                    al1 = softmax_half(ef1, rc1)
                    # alT: transpose unnormalized alpha -> item at col 32j
                    for alx, alTx in ((al0, alT0), (al1, alT1)):
                        pta = ptrans.tile([128, KT_H, 128], F32, tag="tr", name="pta")
                        for rt_ in range(RT):
                            nc.tensor.transpose(
                                pta[:, rt_, :],
                                alx[:, rt_ * 128 : (rt_ + 1) * 128],
                                ident,
                            )
                        nc.vector.tensor_copy(alTx, pta[:, :RT, :])
                    # context (unnormalized): item b -> tile b//4, row 32*(b%4)
                    pc0 = pattn.tile([128, H], F32, tag="pa", name="pc0")
                    pc1 = pattn.tile([128, H], F32, tag="pa", name="pc1")
                    nc.vector.memset(pc0, 0.0)
                    nc.vector.memset(pc1, 0.0)
                    for b_ in range(BS):
                        ps = pc0 if b_ < 4 else pc1
                        j = (b_ % 4) * 32
                        alTx = alT0 if b_ < 4 else alT1
                        for rt_ in range(RT):
                            nc.tensor.matmul(
                                ps[j : j + 1, :],
                                lhsT=alTx[:, rt_, j : j + 1],
                                rhs=sb_hp[:, rt_ * BS + b_, :],
                                start=(rt_ == 0),
                                stop=(rt_ == RT - 1),
                                tile_position=(0, j),
                            )
                    # normalize into strided context tiles
                    nc.vector.tensor_scalar_mul(cstr0, pc0, rc0)
                    nc.vector.tensor_scalar_mul(cstr1, pc1, rc1)
                    # cT: item b at col PB[b] (32j from cstr0, 16+32j from cstr1)
                    for csx, sh in ((cstr0, 0), (cstr1, 16)):
                        ptc = ptrans.tile([128, KT_H, 128], F32, tag="tr", name="ptc")
                        for chk in range(KT_H):
                            nc.tensor.transpose(
                                ptc[:, chk, :],
                                csx[:, chk * 128 : (chk + 1) * 128],
                                ident,
                            )
                        src = ptc.rearrange("p k (i s) -> p k i s", s=32)[:, :, :, 0:1]
                        dst = cT.rearrange("p k (i s) -> p k i s", s=32)[
                            :, :, :, sh : sh + 1
                        ]
                        nc.vector.tensor_copy(dst, src)

                def lstm_cell(gact, cs, s_out):
                    i_s = gact[:, 0:H]
                    f_s = gact[:, H : 2 * H]
                    o_s = gact[:, 2 * H : 3 * H]
                    g_s = gact[:, 3 * H : 4 * H]
                    ig = work.tile([128, H], F32, tag="ig")
                    nc.vector.tensor_mul(ig, i_s, g_s)
                    nc.vector.tensor_mul(cs, f_s, cs)
                    nc.vector.tensor_add(cs, cs, ig)
                    tch = work.tile([128, H], F32, tag="tch")
                    nc.scalar.activation(tch, cs, AF.Tanh)
                    nc.vector.tensor_mul(s_out, o_s, tch)

                def transpose_state(src, dstT):
                    pt = ptrans.tile([128, KT_H, 128], F32, tag="tr", name="ptc2")
                    for chk in range(KT_H):
                        nc.tensor.transpose(
                            pt[:, chk, :], src[:, chk * 128 : (chk + 1) * 128], ident
                        )
                    nc.vector.tensor_copy(dstT, pt)

                def _compact(lhsT_x, kt):
                    # [128, kt, 128] -> [128, 32] (every 4th col; items land at
                    # compact positions 4m so the M=32 matmul writes the whole
                    # 32-row group -- no uninitialized psum rows)
                    return lhsT_x[:, kt, :].rearrange("p (i s) -> p i s", s=4)[
                        :, :, 0
                    ]

                def gates(lhsT_first, lhsT_second, W, bias_tile, gact):
                    """gact = act(W[:512rows].lhsT_first + W[512:].lhsT_second + b).

                    The 8 k-tiles are packed 4-wide across PE column groups
                    (2 accumulation rounds); the four 32-strided partial rows
                    are then summed by one matmul against the host-built
                    selection matrix S (which also lands items on their PB
                    partitions).
                    """
                    for ch in range(4):
                        ps = pgate.tile([128, 512], F32, tag="pg", name="pg")
                        for kt in range(KT_H):
                            nc.tensor.matmul(
                                ps,
                                lhsT=lhsT_second[:, kt, :],
                                rhs=W[:, KT_H + kt, ch * 512 : (ch + 1) * 512],
                                start=(kt == 0),
                                stop=False,
                            )
                        for kt in range(KT_H):
                            nc.tensor.matmul(
                                ps,
                                lhsT=lhsT_first[:, kt, :],
                                rhs=W[:, kt, ch * 512 : (ch + 1) * 512],
                                start=False,
                                stop=(kt == KT_H - 1),
                            )
                        gsl = gact[:, ch * 512 : (ch + 1) * 512]
                        nc.vector.tensor_add(
                            gsl, ps, bias_tile[:, ch * 512 : (ch + 1) * 512]
                        )
                        nc.scalar.activation(
                            gsl, gsl, AF.Sigmoid if ch < 3 else AF.Tanh
                        )

                # ---------------- Phase 2 ----------------
                attention(-1)  # c_init

                for t in range(T_steps):
                    g0a = workbig.tile([128, G], F32, tag="g0a")
                    gates(cT, s0T, sb_W0T, ring[t % 3], g0a)
                    lstm_cell(g0a, cs0, s0)
                    transpose_state(s0, s0T)

                    g1a = workbig.tile([128, G], F32, tag="g1a")
                    gates(s0T, s1T, sb_W1T, sb_b1, g1a)
                    lstm_cell(g1a, cs1, s1)
                    transpose_state(s1, s1T)

                    # spT = (phi_w/sqrt(H)) @ s1, cols in PERM16 item order
                    ptsp = ptrans.tile([128, KT_H, 128], F32, tag="tr", name="ptsp")
                    s1T16 = s1T.rearrange("p k (i s) -> p k i s", s=16)[:, :, :, 0]
                    for mt in range(KT_H):
                        for kt in range(KT_H):
                            nc.tensor.matmul(
                                ptsp[:, mt, :BS],
                                lhsT=sb_phiT[:, kt, mt * 128 : (mt + 1) * 128],
                                rhs=s1T16[:, kt, :],
                                start=(kt == 0),
                                stop=(kt == KT_H - 1),
                            )
                    nc.vector.tensor_copy(spT, ptsp[:, :, :BS])

                    attention(t)

                    nc.sync.dma_start(
                        d_histT[t, :, 0:KT_H, :],
                        s1T.rearrange("p k (i s) -> p k i s", s=16)[:, :, :, 0],
                    )
                    nc.sync.dma_start(
                        d_histT[t, :, KT_H : 2 * KT_H, :],
                        cT.rearrange("p k (i s) -> p k i s", s=16)[:, :, :, 0],
                    )
                    if t + 2 < T_steps:
                        nc.sync.dma_start(_s16(ring[(t + 2) % 3]), d_embb[t + 2])

                for cm in reversed(_p2cms):
                    cm.__exit__(None, None, None)

            # ---------------- Phase 3: output projection ----------------
            with (
                tc.tile_pool(name="ph3", bufs=1) as ph3,
                tc.tile_pool(name="ph3w", bufs=2) as ph3w,
                tc.tile_pool(name="pp3", bufs=4, space="PSUM") as pp3,
            ):
                NTB = T_steps * BS
                sb_hist = ph3.tile([128, KT_KC, NTB], F16)
                hist_v = d_histT.rearrange("t p k b -> p k t b")
                for kt in range(KT_KC):
                    nc.sync.dma_start(
                        sb_hist[:, kt, :].rearrange("p (t b) -> p t b", b=BS),
                        hist_v[:, kt, :, :],
                    )
                sb_ob = ph3.tile([128, V], F32)
                nc.sync.dma_start(sb_ob, d_ob_bc[:])
                out_tb = d_out.rearrange("t b v -> (t b) v")
                owT_v = d_owT.rearrange("(k p) v -> p k v", p=128)
                for nch in range(V // 512):  # 8
                    rhs = ph3w.tile([128, KT_KC, 512], F16, tag="owr", name="owr")
                    for kt in range(KT_KC):
                        nc.sync.dma_start(
                            rhs[:, kt, :], owT_v[:, kt, nch * 512 : (nch + 1) * 512]
                        )
                    for m in range(NTB // 128):
                        ps = pp3.tile([128, 512], F32, tag="po", name="po")
                        for kt in range(KT_KC):
                            nc.tensor.matmul(
                                ps,
                                lhsT=sb_hist[:, kt, m * 128 : (m + 1) * 128],
                                rhs=rhs[:, kt, :],
                                start=(kt == 0),
                                stop=(kt == KT_KC - 1),
                            )
                        ost = ph3w.tile([128, 512], F32, tag="ost", name="ost")
                        nc.vector.tensor_add(
                            ost, ps, sb_ob[:, nch * 512 : (nch + 1) * 512]
                        )
                        nc.sync.dma_start(
                            out_tb[m * 128 : (m + 1) * 128, nch * 512 : (nch + 1) * 512],
                            ost,
                        )
    nc.compile()
    return nc


def host_prep(inputs, T_steps=T):
    f = lambda k: np.asarray(inputs[k], np.float32)
    h = f("h")
    y = np.asarray(inputs["y"])
    w_ih0, w_hh0 = f("w_ih0"), f("w_hh0")
    b_ih0, b_hh0 = f("b_ih0"), f("b_hh0")
    w_ih1, w_hh1 = f("w_ih1"), f("w_hh1")
    b_ih1, b_hh1 = f("b_ih1"), f("b_hh1")
    phi_w, phi_b = f("phi_w"), f("phi_b")
    psi_w, psi_b = f("psi_w"), f("psi_b")
    out_w, out_b = f("out_w"), f("out_b")

    scale = 1.0 / math.sqrt(H)
    # gate reorder i,f,g,o -> i,f,o,g
    perm = np.concatenate(
        [np.arange(H), H + np.arange(H), 3 * H + np.arange(H), 2 * H + np.arange(H)]
    )
    W0T = np.concatenate([w_ih0[:, V:], w_hh0], axis=1)[perm].T  # [1024, 2048]
    W1T = np.concatenate([w_ih1, w_hh1], axis=1)[perm].T
    phiT = (phi_w * scale).T
    psiT = psi_w.T
    psibT = np.ascontiguousarray(psi_b.reshape(KT_H, 128).T)
    psib_bc = np.ascontiguousarray(np.tile(psi_b[None, :], (128, 1)))
    phibT = np.ascontiguousarray(
        ((phi_b * scale).reshape(KT_H, 128).T).astype(np.float16)
    )
    b1_bc = np.ascontiguousarray(np.tile((b_ih1 + b_hh1)[perm][None, :], (128, 1)))
    b0 = (b_ih0 + b_hh0)[perm]
    embW = w_ih0[:, :V][perm]  # [2048, 4096]
    embb_all = embW.T[y[:, :T_steps]] + b0  # [B, T, 2048]
    embb_all = np.ascontiguousarray(embb_all.transpose(1, 0, 2))  # [T, B, 2048]
    owT = out_w.T
    ob_bc = np.ascontiguousarray(np.tile(out_b[None, :], (128, 1)))
    smat = np.zeros((128, 128), np.float16)
    for j in range(4):
        for m in range(BS):
            smat[32 * j + 4 * m, 16 * m] = 1.0

    c16 = lambda x: np.ascontiguousarray(x.astype(np.float16))
    shared = dict(
        W0T=c16(W0T), W1T=c16(W1T), phiT=c16(phiT), psiT=c16(psiT),
        psib_bc=psib_bc, psibT=psibT, phibT=phibT,
        b1_bc=b1_bc, owT=c16(owT), ob_bc=ob_bc, smat=smat,
    )
    in_maps = []
    for ci in range(NCORES):
        sl = slice(ci * BS, (ci + 1) * BS)
        m = dict(shared)
        m["hT"] = c16(h[sl].reshape(BS * R, H).T)
        m["embb"] = np.ascontiguousarray(embb_all[:, sl, :][:, PERM16, :])
        in_maps.append(m)
    return in_maps


def gather_output(per_core_outs):
    """per-core device outs [T, 8(PERM16 order), V] -> [B, T, V]."""
    shards = []
    for o in per_core_outs:
        shards.append(np.ascontiguousarray(o[:, INV16, :].transpose(1, 0, 2)))
    return np.concatenate(shards, axis=0)


def kernel(**inputs):
    nc = build_program(T)
    in_maps = host_prep(inputs, T)
    res = run_bass_kernel_spmd(nc, in_maps, list(range(NCORES)))
    out = gather_output([res.results[ci]["out"] for ci in range(NCORES)])
    return np.ascontiguousarray(out.astype(np.float32))



# revision 4
# speedup vs baseline: 2.3468x; 2.3468x over previous
"""AttendAndSpell v5a Trainium2 Bass kernel, v3.

vs v2: items live at rows/cols 0-7 (no strided packing); every transpose is
replaced by an N=8 selection matmul; all-tanh activations (sigmoid via
host-halved weights + fused (x+1)*0.5 on DVE) so the activation table is
loaded once; one [8, 2048] tanh per gates call; fp16 gate activations for
2x DVE; spT via dual-fp8; persistent psum tiles with one-time memsets.

Item b on partition row b (states) / column b (transposed stationaries).
Attention psums still pack 4 items per bank at rows 32j via tile_position.
"""

import math

import numpy as np
import ml_dtypes

import concourse.bacc as bacc
import concourse.mybir as mybir
import concourse.tile as tile
from concourse.bass_utils import run_bass_kernel_spmd

B, R, T, H, V = 64, 256, 128, 512, 4096
NCORES = 8
BS = B // NCORES  # 8
G = 4 * H  # 2048
KC = 2 * H  # 1024
KT_H = H // 128  # 4
KT_KC = KC // 128  # 8
RT = R // 128  # 2
F32 = mybir.dt.float32
F16 = mybir.dt.float16
F8 = mybir.dt.float8e4
AF = mybir.ActivationFunctionType
ALU = mybir.AluOpType
AX = mybir.AxisListType
PM = mybir.MatmulPerfMode
E4 = ml_dtypes.float8_e4m3


def build_program(T_steps=T):
    nc = bacc.Bacc(None, target_bir_lowering=False)

    d_hT = nc.dram_tensor("hT", [H, BS * R], F16, kind="ExternalInput")
    d_W08 = nc.dram_tensor("W08", [128, KT_KC, G], F8, kind="ExternalInput")
    d_W18 = nc.dram_tensor("W18", [128, KT_KC, G], F8, kind="ExternalInput")
    d_phiT8 = nc.dram_tensor("phiT8", [128, KT_H, KT_H, 128], F8,
                             kind="ExternalInput")
    d_phibT16 = nc.dram_tensor("phibT16", [1, KT_H, 128], F16,
                               kind="ExternalInput")
    d_psiT = nc.dram_tensor("psiT", [H, H], F16, kind="ExternalInput")
    d_psib_bc = nc.dram_tensor("psib_bc", [128, H], F32, kind="ExternalInput")
    d_psibT = nc.dram_tensor("psibT", [128, KT_H], F32, kind="ExternalInput")
    d_phibT = nc.dram_tensor("phibT", [128, KT_H], F32, kind="ExternalInput")
    d_ident8 = nc.dram_tensor("ident8", [BS, BS], F16, kind="ExternalInput")
    d_ones128 = nc.dram_tensor("ones128", [128, 1], F16, kind="ExternalInput")
    d_sel832a = nc.dram_tensor("sel832a", [BS, 128], F16, kind="ExternalInput")
    d_sel832b = nc.dram_tensor("sel832b", [BS, 128], F16, kind="ExternalInput")
    d_sel32a = nc.dram_tensor("sel32a", [128, BS], F16, kind="ExternalInput")
    d_sel32b = nc.dram_tensor("sel32b", [128, BS], F16, kind="ExternalInput")
    d_ones8 = nc.dram_tensor("ones8", [1, BS], F16, kind="ExternalInput")
    d_b1bc = nc.dram_tensor("b1bc", [BS, G], F16, kind="ExternalInput")
    d_embb = nc.dram_tensor("embb", [T_steps, BS, G], F16, kind="ExternalInput")
    d_owT = nc.dram_tensor("owT", [KC, V], F16, kind="ExternalInput")
    d_ob_bc = nc.dram_tensor("ob_bc", [128, V], F32, kind="ExternalInput")
    d_out = nc.dram_tensor("out", [T_steps, BS, V], F32, kind="ExternalOutput")
    d_histT = nc.dram_tensor("histT", [T_steps, 128, 2 * KT_H, BS], F16)

    with tile.TileContext(nc) as tc:
        with (
            tc.tile_pool(name="persist", bufs=1) as persist,
            tc.tile_pool(name="work", bufs=2) as work,
        ):
            # small host consts
            sb_ident8 = persist.tile([BS, BS], F16)
            nc.sync.dma_start(sb_ident8, d_ident8[:])
            sb_ones128 = persist.tile([128, 1], F16)
            nc.sync.dma_start(sb_ones128, d_ones128[:])
            sb_sel832a = persist.tile([BS, 128], F16)
            nc.sync.dma_start(sb_sel832a, d_sel832a[:])
            sb_sel832b = persist.tile([BS, 128], F16)
            nc.sync.dma_start(sb_sel832b, d_sel832b[:])
            sb_sel32a = persist.tile([128, BS], F16)
            nc.sync.dma_start(sb_sel32a, d_sel32a[:])
            sb_sel32b = persist.tile([128, BS], F16)
            nc.sync.dma_start(sb_sel32b, d_sel32b[:])
            sb_ones8 = persist.tile([1, BS], F16)
            nc.sync.dma_start(sb_ones8, d_ones8[:])
            sb_phibT16 = persist.tile([1, KT_H, 128], F16)
            nc.sync.dma_start(sb_phibT16, d_phibT16[:])
            sb_psibT = persist.tile([128, KT_H], F32)
            nc.sync.dma_start(sb_psibT, d_psibT[:])
            sb_phibT = persist.tile([128, KT_H], F32)
            nc.sync.dma_start(sb_phibT, d_phibT[:])
            sb_b1bc = persist.tile([BS, G], F16)
            nc.sync.dma_start(sb_b1bc, d_b1bc[:])
            zeros8 = persist.tile([128, BS], F32)
            nc.vector.memset(zeros8, 0.0)

            # fp32 cell states, items on partitions 0-7
            cs0 = persist.tile([BS, H], F32)
            cs1 = persist.tile([BS, H], F32)
            nc.vector.memset(cs0, 0.0)
            nc.vector.memset(cs1, 0.0)
            # fp8 transposed stationaries; item b at col b, cols 8-127 zero
            s0T8 = persist.tile([128, KT_H, 128], F8)
            s1T8 = persist.tile([128, KT_H, 128], F8)
            cT8 = persist.tile([128, KT_H, 128], F8)
            nc.vector.memset(s0T8, 0.0)
            nc.vector.memset(s1T8, 0.0)
            nc.vector.memset(cT8, 0.0)
            spT8 = persist.tile([128, KT_H, BS], F8)
            alT16 = persist.tile([128, RT, BS], F16)
            rc8sb = persist.tile([BS, 1], F16)
            rc32 = persist.tile([128, 2], F32)
            s1T16h = persist.tile([128, 2 * KT_H, BS], F16)  # hist: s1T | cT

            ring = [
                persist.tile([BS, G], F16, name=f"ring{i}", tag=f"ring{i}")
                for i in range(3)
            ]

            with tc.tile_pool(name="wts", bufs=1) as wts:
                sb_W08 = wts.tile([128, KT_KC, G], F8)
                nc.sync.dma_start(sb_W08, d_W08[:])
                sb_W18 = wts.tile([128, KT_KC, G], F8)
                nc.sync.dma_start(sb_W18, d_W18[:])
                sb_phiT8 = wts.tile([128, KT_H, KT_H, 128], F8)
                nc.sync.dma_start(sb_phiT8, d_phiT8[:])
                sb_hp = wts.tile([128, RT * BS, H], F16)  # [p, rt*BS+b, d]
                # hpT8[p, b, dpair, dsub, rchunk, 128]: dual-fp8 scores stationary
                sb_hpT8 = wts.tile([128, BS, 2, 2, RT, 128], F8)

                # ---------------- Phase 1: hp / hpT ----------------
                with (
                    tc.tile_pool(name="ph1", bufs=1) as ph1,
                    tc.tile_pool(name="pp1", bufs=2, space="PSUM") as pp1,
                ):
                    NBR = BS * R  # 2048
                    sb_hT = ph1.tile([128, KT_H, NBR], F16)
                    nc.sync.dma_start(sb_hT, d_hT.rearrange("(kt p) n -> p kt n", p=128))
                    sb_psiT = ph1.tile([128, KT_H, H], F16)
                    nc.sync.dma_start(
                        sb_psiT, d_psiT.rearrange("(kt p) f -> p kt f", p=128)
                    )
                    sb_psib = ph1.tile([128, H], F32)
                    nc.sync.dma_start(sb_psib, d_psib_bc[:])

                    for m in range(NBR // 128):  # 16
                        ps = pp1.tile([128, H], F32, tag="pp1", name="ps1")
                        for kt in range(KT_H):
                            nc.tensor.matmul(
                                ps,
                                lhsT=sb_hT[:, kt, m * 128 : (m + 1) * 128],
                                rhs=sb_psiT[:, kt, :],
                                start=(kt == 0),
                                stop=(kt == KT_H - 1),
                            )
                        b_, rt_ = divmod(m, RT)
                        nc.vector.tensor_add(sb_hp[:, rt_ * BS + b_, :], ps, sb_psib)
                    for mt in range(KT_H):
                        for nch in range(NBR // 512):  # 4
                            ps = pp1.tile([128, H], F32, tag="pp1", name="ps2")
                            for kt in range(KT_H):
                                nc.tensor.matmul(
                                    ps,
                                    lhsT=sb_psiT[:, kt, mt * 128 : (mt + 1) * 128],
                                    rhs=sb_hT[:, kt, nch * 512 : (nch + 1) * 512],
                                    start=(kt == 0),
                                    stop=(kt == KT_H - 1),
                                )
                            for j in range(512 // R):
                                b_ = nch * 2 + j
                                nc.vector.tensor_scalar_add(
                                    sb_hpT8[:, b_, mt // 2, mt % 2, :, :],
                                    ps[:, j * R : (j + 1) * R].rearrange(
                                        "p (rt r) -> p rt r", r=128
                                    ),
                                    sb_psibT[:, mt : mt + 1],
                                )

                _p2cms = [
                    tc.tile_pool(name="pg", bufs=1, space="PSUM"),
                    tc.tile_pool(name="pa", bufs=1, space="PSUM"),
                    tc.tile_pool(name="psel", bufs=1, space="PSUM"),
                ]
                pg, pa, psel = [cm.__enter__() for cm in _p2cms]

                # persistent psum tiles (memset ONCE; unwritten rows stay 0)
                ps_g = [
                    pg.tile([128, 512], F32, tag=f"g{c}", name=f"ps_g{c}")
                    for c in range(4)
                ]
                ps_sc = pa.tile([128, 512], F32, tag="sc", name="ps_sc")
                ps_c0 = pa.tile([128, 512], F32, tag="c0", name="ps_c0")
                ps_c1 = pa.tile([128, 512], F32, tag="c1", name="ps_c1")
                # one shared bank for the four small select psums; only the
                # state_selects use start=True (whole-bank pending), the other
                # groups rely on pending-zero from the most recent start.
                ps_small = psel.tile([128, 16, KT_H, BS], F32, tag="sm", name="ps_small")
                ps_st = ps_small[:, 0]
                ps_ct = ps_small[:, 2]
                ps_sp = ps_small[:, 3]
                ps_e = ps_small[:, 4, 0:RT, :]
                ps_sum = ps_small[0:BS, 5, 0, 0:1]
                ps_rc = ps_small[:, 5, 1, 0:2]
                for p in (*ps_g, ps_sc, ps_c0, ps_c1, ps_small):
                    nc.vector.memset(p, 0.0)

                for tpre in range(min(2, T_steps)):
                    nc.sync.dma_start(ring[tpre], d_embb[tpre])

                def lstm_cell(gact8, cs, s16):
                    g_s = gact8[:, 0:H]
                    i_s = gact8[:, H : 2 * H]
                    f_s = gact8[:, 2 * H : 3 * H]
                    o_s = gact8[:, 3 * H : 4 * H]
                    ig = work.tile([BS, H], F16, tag="ig", name="ig")
                    nc.vector.tensor_mul(ig, i_s, g_s)
                    fc = work.tile([BS, H], F32, tag="fc", name="fc")
                    nc.vector.tensor_mul(fc, f_s, cs)
                    nc.vector.tensor_add(cs, fc, ig)
                    tch = work.tile([BS, H], F16, tag="tch", name="tch")
                    nc.scalar.activation(tch, cs, AF.Tanh)
                    nc.vector.tensor_mul(s16, o_s, tch)

                def state_select(s16):
                    """s16 [8, 512] -> ps_st [128, kt, 8] (s16 chunks^T)."""
                    for kt in range(KT_H):
                        nc.tensor.matmul(
                            ps_st[:, kt, :],
                            lhsT=s16[:, kt * 128 : (kt + 1) * 128],
                            rhs=sb_ident8,
                            start=(kt == 0), stop=(kt == KT_H - 1),
                            skip_group_check=True,
                        )

                def attention(t):
                    # eT scores: stationary hpT8 dual pairs, moving spT8 col
                    # -> ps_e[128 r-chunk, rt, b]; all rows valid.
                    for b_ in range(BS):
                        for rc_ in range(RT):
                            for p in range(2):
                                nc.tensor.matmul(
                                    ps_e[:, rc_, b_ : b_ + 1],
                                    lhsT=sb_hpT8[:, b_, p, :, rc_, :],
                                    rhs=spT8[:, 2 * p : 2 * p + 2, b_ : b_ + 1],
                                    start=False, stop=(p == 1),
                                    perf_mode=PM.DoubleRow,
                                    skip_group_check=True,
                                )
                    # alpha = exp(e) (no max-sub; e in [-1.5, 1.5])
                    nc.scalar.activation(alT16, ps_e, AF.Exp)
                    # row sums via ones matmul -> [8, 1]; rc to rows 32j
                    for rt_ in range(RT):
                        nc.tensor.matmul(
                            ps_sum,
                            lhsT=alT16[:, rt_, :],
                            rhs=sb_ones128,
                            start=False, stop=(rt_ == RT - 1),
                            skip_group_check=True,
                        )
                    with nc.allow_low_precision(reason="rc fp16 for matmul"):
                        nc.vector.reciprocal(rc8sb, ps_sum)
                    nc.tensor.matmul(
                        ps_rc[:, 0:1], lhsT=sb_sel832a, rhs=rc8sb,
                        start=False, stop=False, skip_group_check=True,
                    )
                    nc.tensor.matmul(
                        ps_rc[:, 1:2], lhsT=sb_sel832b, rhs=rc8sb,
                        start=False, stop=True, skip_group_check=True,
                    )
                    nc.vector.tensor_copy(rc32, ps_rc)
                    # context with unnormalized alpha; rows 32j per group
                    for b_ in range(BS):
                        gq, j = (0, 32 * b_) if b_ < 4 else (1, 32 * (b_ - 4))
                        pc = ps_c0 if b_ < 4 else ps_c1
                        for rt_ in range(RT):
                            nc.tensor.matmul(
                                pc[j : j + 1, :],
                                lhsT=alT16[:, rt_, b_ : b_ + 1],
                                rhs=sb_hp[:, rt_ * BS + b_, :],
                                start=(rt_ == 0),
                                stop=(rt_ == RT - 1),
                                tile_position=(0, j),
                                skip_group_check=True,
                            )
                    # normalize -> fp16; select into ps_ct [128, kt, 8]
                    c16a = work.tile([128, H], F16, tag="c16a", name="c16a")
                    c16b = work.tile([128, H], F16, tag="c16b", name="c16b")
                    nc.vector.tensor_scalar_mul(c16a, ps_c0, rc32[:, 0:1])
                    nc.scalar.activation(c16b, ps_c1, AF.Copy, scale=rc32[:, 1:2])
                    for kt in range(KT_H):
                        sl = slice(kt * 128, (kt + 1) * 128)
                        nc.tensor.matmul(
                            ps_ct[:, kt, :], lhsT=c16a[:, sl], rhs=sb_sel32a,
                            start=False, stop=False,
                            skip_group_check=True,
                        )
                        nc.tensor.matmul(
                            ps_ct[:, kt, :], lhsT=c16b[:, sl], rhs=sb_sel32b,
                            start=False, stop=(kt == KT_H - 1),
                            skip_group_check=True,
                        )
                    nc.vector.tensor_copy(cT8[:, :, 0:BS], ps_ct)
                    if t >= 0:
                        nc.vector.tensor_copy(s1T16h[:, KT_H:, :], ps_ct)

                # ---------------- c_init ----------------
                for mt in range(KT_H):
                    nc.vector.tensor_scalar_add(
                        spT8[:, mt, :], zeros8, sb_phibT[:, mt : mt + 1]
                    )
                attention(-1)

                # ---------------- Phase 2: decode loop ----------------
                def gates_half(lhsT_b, W8, start):
                    """Issue the recurrent-state half of a gates call."""
                    for ch in range(4):
                        sl = slice(ch * 512, (ch + 1) * 512)
                        for p in range(2):
                            nc.tensor.matmul(
                                ps_g[ch],
                                lhsT=lhsT_b[:, 2 * p : 2 * p + 2, :],
                                rhs=W8[:, 4 + 2 * p : 4 + 2 * p + 2, sl],
                                start=(start and p == 0), stop=False,
                                perf_mode=PM.DoubleRow,
                                skip_group_check=True,
                            )

                def gates_fin(bias_rhs, lhsT_a, W8, gact8):
                    """First half of x (cT or s0T) + bias + act; the lhsT_b
                    half must already be accumulated in ps_g."""
                    pre = work.tile([BS, G], F16, tag="gpre", name="gpre")
                    for ch in range(4):
                        sl = slice(ch * 512, (ch + 1) * 512)
                        for p in range(2):
                            nc.tensor.matmul(
                                ps_g[ch],
                                lhsT=lhsT_a[:, 2 * p : 2 * p + 2, :],
                                rhs=W8[:, 2 * p : 2 * p + 2, sl],
                                start=False, stop=(p == 1),
                                perf_mode=PM.DoubleRow,
                                skip_group_check=True,
                            )
                        with nc.allow_low_precision(reason="tanh input fp16"):
                            nc.vector.tensor_add(
                                pre[:, sl], ps_g[ch][0:BS, :], bias_rhs[:, sl]
                            )
                        nc.scalar.activation(gact8[:, sl], pre[:, sl], AF.Tanh)
                        if ch > 0:
                            nc.gpsimd.tensor_scalar(
                                gact8[:, sl], gact8[:, sl], 1.0, 0.5,
                                ALU.add, ALU.mult,
                            )

                for t in range(T_steps):
                    gates_half(s0T8, sb_W08, True)
                    g0a = work.tile([BS, G], F16, tag="g0a", name="g0a")
                    gates_fin(ring[t % 3], cT8, sb_W08, g0a)
                    # gates1's s1-half streams while lstm0 runs on DVE/Act
                    gates_half(s1T8, sb_W18, True)
                    s016 = work.tile([BS, H], F16, tag="s016", name="s016")
                    lstm_cell(g0a, cs0, s016)
                    state_select(s016)
                    nc.vector.tensor_copy(s0T8[:, :, 0:BS], ps_st)

                    g1a = work.tile([BS, G], F16, tag="g1a", name="g1a")
                    gates_fin(sb_b1bc, s0T8, sb_W18, g1a)
                    s116 = work.tile([BS, H], F16, tag="s116", name="s116")
                    lstm_cell(g1a, cs1, s116)
                    state_select(s116)
                    nc.vector.tensor_copy(s1T8[:, :, 0:BS], ps_st)
                    nc.vector.tensor_copy(s1T16h[:, 0:KT_H, :], ps_st)

                    # spT = phiT8 @ s1T8 (dual) + phib (K=1 matmul)
                    for mt in range(KT_H):
                        nc.tensor.matmul(
                            ps_sp[:, mt, :],
                            lhsT=sb_phibT16[:, mt, :],
                            rhs=sb_ones8,
                            start=False, stop=False,
                            skip_group_check=True,
                        )
                    for mt in range(KT_H):
                        for p in range(2):
                            nc.tensor.matmul(
                                ps_sp[:, mt, :],
                                lhsT=sb_phiT8[:, mt, 2 * p : 2 * p + 2, :],
                                rhs=s1T8[:, 2 * p : 2 * p + 2, 0:BS],
                                start=False,
                                stop=(mt == KT_H - 1 and p == 1),
                                perf_mode=PM.DoubleRow,
                                skip_group_check=True,
                            )
                    nc.vector.tensor_copy(spT8, ps_sp)

                    attention(t)

                    nc.sync.dma_start(d_histT[t], s1T16h)
                    if t + 2 < T_steps:
                        nc.sync.dma_start(ring[(t + 2) % 3], d_embb[t + 2])

                for cm in reversed(_p2cms):
                    cm.__exit__(None, None, None)

            # ---------------- Phase 3: output projection ----------------
            with (
                tc.tile_pool(name="ph3", bufs=1) as ph3,
                tc.tile_pool(name="ph3w", bufs=2) as ph3w,
                tc.tile_pool(name="pp3", bufs=4, space="PSUM") as pp3,
            ):
                NTB = T_steps * BS
                sb_hist = ph3.tile([128, KT_KC, NTB], F16)
                hist_v = d_histT.rearrange("t p k b -> p k t b")
                for kt in range(KT_KC):
                    nc.sync.dma_start(
                        sb_hist[:, kt, :].rearrange("p (t b) -> p t b", b=BS),
                        hist_v[:, kt, :, :],
                    )
                sb_ob = ph3.tile([128, V], F32)
                nc.sync.dma_start(sb_ob, d_ob_bc[:])
                out_tb = d_out.rearrange("t b v -> (t b) v")
                owT_v = d_owT.rearrange("(k p) v -> p k v", p=128)
                for nch in range(V // 512):  # 8
                    rhs = ph3w.tile([128, KT_KC, 512], F16, tag="owr", name="owr")
                    for kt in range(KT_KC):
                        nc.sync.dma_start(
                            rhs[:, kt, :], owT_v[:, kt, nch * 512 : (nch + 1) * 512]
                        )
                    for m in range(NTB // 128):
                        ps = pp3.tile([128, 512], F32, tag="po", name="po")
                        for kt in range(KT_KC):
                            nc.tensor.matmul(
                                ps,
                                lhsT=sb_hist[:, kt, m * 128 : (m + 1) * 128],
                                rhs=rhs[:, kt, :],
                                start=(kt == 0),
                                stop=(kt == KT_KC - 1),
                            )
                        ost = ph3w.tile([128, 512], F32, tag="ost", name="ost")
                        nc.vector.tensor_add(
                            ost, ps, sb_ob[:, nch * 512 : (nch + 1) * 512]
                        )
                        nc.sync.dma_start(
                            out_tb[m * 128 : (m + 1) * 128, nch * 512 : (nch + 1) * 512],
                            ost,
                        )
    nc.compile()
    return nc


def host_prep(inputs, T_steps=T):
    f = lambda k: np.asarray(inputs[k], np.float32)
    h = f("h")
    y = np.asarray(inputs["y"])
    w_ih0, w_hh0 = f("w_ih0"), f("w_hh0")
    b_ih0, b_hh0 = f("b_ih0"), f("b_hh0")
    w_ih1, w_hh1 = f("w_ih1"), f("w_hh1")
    b_ih1, b_hh1 = f("b_ih1"), f("b_hh1")
    phi_w, phi_b = f("phi_w"), f("phi_b")
    psi_w, psi_b = f("psi_w"), f("psi_b")
    out_w, out_b = f("out_w"), f("out_b")

    scale = 1.0 / math.sqrt(H)
    # gate chunk order g,i,f,o; i,f,o pre-scaled by 0.5 (tanh sigmoid)
    perm = np.concatenate(
        [2 * H + np.arange(H), np.arange(H), H + np.arange(H), 3 * H + np.arange(H)]
    )
    gsc = np.concatenate([np.ones(H, np.float32), np.full(3 * H, 0.5, np.float32)])
    c8 = lambda x: np.ascontiguousarray(x.astype(E4))
    c16 = lambda x: np.ascontiguousarray(x.astype(np.float16))

    def w8fmt(Wcat):
        WT = (Wcat[perm] * gsc[:, None]).T  # [KC, G]
        return c8(WT.reshape(KT_KC, 128, G).transpose(1, 0, 2))

    W08 = w8fmt(np.concatenate([w_ih0[:, V:], w_hh0], axis=1))
    W18 = w8fmt(np.concatenate([w_ih1, w_hh1], axis=1))

    phiS = phi_w * scale  # [H(out d), H(in d)]
    # phiT8[p, mt, kt, m] = phiS[mt*128+m, kt*128+p]
    phiT8 = c8(phiS.reshape(KT_H, 128, KT_H, 128).transpose(3, 0, 2, 1))
    phibT16 = c16((phi_b * scale).reshape(1, KT_H, 128))
    phibT = np.ascontiguousarray((phi_b * scale).reshape(KT_H, 128).T)
    psiT = psi_w.T
    psibT = np.ascontiguousarray(psi_b.reshape(KT_H, 128).T)
    psib_bc = np.ascontiguousarray(np.tile(psi_b[None, :], (128, 1)))

    ident8 = np.eye(BS, dtype=np.float16)
    ones128 = np.ones((128, 1), np.float16)
    sel832a = np.zeros((BS, 128), np.float16)
    sel832b = np.zeros((BS, 128), np.float16)
    for j in range(4):
        sel832a[j, 32 * j] = 1.0
        sel832b[4 + j, 32 * j] = 1.0
    sel32a = np.zeros((128, BS), np.float16)
    sel32b = np.zeros((128, BS), np.float16)
    for i in range(4):
        sel32a[32 * i, i] = 1.0
        sel32b[32 * i, 4 + i] = 1.0
    ones8 = np.ones((1, BS), np.float16)

    b1bc = np.tile(((b_ih1 + b_hh1)[perm] * gsc)[None, :], (BS, 1))
    b0 = (b_ih0 + b_hh0)[perm]
    embW = w_ih0[:, :V][perm]  # [2048, 4096]
    embb_all = (embW.T[y[:, :T_steps]] + b0) * gsc  # [B, T, 2048]
    embb_all = np.ascontiguousarray(embb_all.transpose(1, 0, 2))  # [T, B, G]
    owT = out_w.T
    ob_bc = np.ascontiguousarray(np.tile(out_b[None, :], (128, 1)))

    shared = dict(
        W08=W08, W18=W18, phiT8=phiT8, phibT16=phibT16, psiT=c16(psiT),
        psib_bc=psib_bc, psibT=psibT, phibT=phibT,
        ident8=ident8, ones128=ones128, sel832a=sel832a, sel832b=sel832b,
        sel32a=sel32a, sel32b=sel32b, ones8=ones8, b1bc=c16(b1bc), owT=c16(owT), ob_bc=ob_bc,
    )
    in_maps = []
    for ci in range(NCORES):
        sl = slice(ci * BS, (ci + 1) * BS)
        m = dict(shared)
        m["hT"] = c16(h[sl].reshape(BS * R, H).T)
        m["embb"] = c16(embb_all[:, sl, :])
        in_maps.append(m)
    return in_maps


def gather_output(per_core_outs):
    """per-core device outs [T, 8, V] -> [B, T, V]."""
    return np.concatenate(
        [np.ascontiguousarray(o.transpose(1, 0, 2)) for o in per_core_outs], axis=0
    )


def kernel(**inputs):
    nc = build_program(T)
    in_maps = host_prep(inputs, T)
    res = run_bass_kernel_spmd(nc, in_maps, list(range(NCORES)))
    out = gather_output([res.results[ci]["out"] for ci in range(NCORES)])
    return np.ascontiguousarray(out.astype(np.float32))
